# revision 26
# baseline (speedup 1.0000x reference)
"""Chamfer rate-distortion loss on 8 TRN2 NeuronCores.

Layout: 8 cores = 4 batches x 2 chamfer directions. Each core computes, for
its (batch, direction), the per-point nearest-neighbor squared distance of
8192 query points X against 8192 reference points Y.

Device algorithm per core:
  - X and Y are pre-sorted (host) along coordinate AXIS.
  - matmul trick (fp16 hi/lo split, K=13, ~1e-6 rel precision):
    PSUM[m,p] = SCALE^2*(|x_m|^2 - 2 x_m.y_p + |y_p|^2) = SCALE^2*D[m,p] >= 0.
    Including |x|^2 via two extra contraction rows keeps PSUM values small
    and non-negative, so fp16 intermediates in the reduction stay accurate
    (overflow to +inf is min-safe).
  - 64 chunks of 128 sorted queries each scan a 192-wide window of sorted Y
    centered on their own sorted position (guard G=32 each side); edges are
    padded with duplicates of the extreme real points (which can never lower
    a min below the true min).
  - The wr input is laid out in 4 column-shards at SBUF partition groups
    {0,32,64,96} (13 rows each) so the HBM load spreads over 52 partition
    lines instead of 13; six interleaved column-slice DMAs split over the SP
    and Scalar queues let the first matmuls start while the bulk streams in.
  - min-reduction is split across two engines: for 7 of 8 chunk-groups the
    ACT engine drains PSUM -> SBUF fp16 and DVE folds in fp16 (2x mode)
    then reduces; the last group is DVE sub-reduces straight from PSUM so
    the post-matmul tail is one small reduce.
    (GPSIMD cannot touch PSUM and its codegen has no min/max TensorTensor;
    DVE TensorTensor may read at most one PSUM operand; tensor_reduce has
    no 2x modes. These constraints shape the split.)

Exactness: for a query x, every Y outside its window differs from x along
the sort axis by at least gap(x), so any excluded point has D >= gap^2.
The host flags points whose Morton-candidate NN upper bound exceeds their
gap ("hard" points, data-dependent but sound) and recomputes them exactly
on the host; a post-hoc check dmin <= gap^2 - margin catches any residual
device noise and falls back to exact host recompute for those rows.
"""

import os

import numpy as np

B, M, P = 4, 8192, 8192
AXIS = 2
SUB = 128            # chunk: 128 sorted queries share one window
GUARD = 32           # guaranteed positions each side beyond the chunk span
BAND = SUB + 2 * GUARD   # 192 window width per chunk
PAD = GUARD          # edge-dup pad; window for chunk u = rt cols [128u, 128u+224)
NBLK = M // SUB      # 64 chunks
GRP = 8              # chunks per PSUM tile / consumer group
NSHARD = 4           # column shards at partition groups 0/32/64/96
NCH_S = 16           # chunks per shard
WTS = M // NSHARD        # 2048 wt cols per shard
RTS = WTS + 2 * PAD      # 2144 rt cols per shard (96-col overlap)
COLS = WTS + RTS         # 4192 cols per shard line
KROWS = 13           # fp16 hi/lo decomposition rows (see _prep_core)
SCALE = 32.0         # coordinate pre-scale; device min is SCALE^2 * real
LMBDA = 5.0
PATHS = "bbbbbbbd"  # b=ACT drain + DVE fp16 folds, d=DVE sub-reduces (last: short tail)

_CACHE = {}
LAST_RESULTS = None


def _build_bass():
    import concourse.tile as tile
    from concourse import bacc, mybir

    nc = bacc.Bacc(None, target_bir_lowering=False, debug=False)
    f32 = mybir.dt.float32
    f16 = mybir.dt.float16
    bf16 = mybir.dt.bfloat16
    MIN = mybir.AluOpType.min
    X = mybir.AxisListType.X

    wr_d = nc.dram_tensor("wr", [128, COLS], f16, kind="ExternalInput")
    out_d = nc.dram_tensor("out", [128, NBLK], f32, kind="ExternalOutput")

    with tile.TileContext(nc) as tc:
        with (
            tc.tile_pool(name="const", bufs=1) as cpool,
            tc.tile_pool(name="outp", bufs=1) as opool,
            tc.tile_pool(name="sba", bufs=2) as apool,
            tc.tile_pool(name="psum", bufs=2, space="PSUM") as ppool,
        ):
            wr = cpool.tile([128, COLS], f16)
            # wt in 4 slices of 512 cols, rt in 4 slices of 536; spread over
            # three DMA queues ordered so chunk 0's inputs (W0+R0) land first
            # rt/wt slices interleaved on one queue so early chunks' inputs
            # land first while later slices stream behind the PE
            WS, RS = WTS // 3, RTS // 3
            for k in range(3):
                nc.sync.dma_start(
                    wr[:, WTS + RS * k:WTS + (RS * (k + 1) if k < 2 else RTS)],
                    wr_d[:, WTS + RS * k:WTS + (RS * (k + 1) if k < 2 else RTS)])
                nc.scalar.dma_start(
                    wr[:, WS * k:WS * (k + 1) if k < 2 else WTS],
                    wr_d[:, WS * k:WS * (k + 1) if k < 2 else WTS])
            outt = opool.tile([128, NBLK], f32)

            H1, H2, H3 = BAND // 2, BAND // 4, BAND // 8  # 96/48/24

            def emit_mms(k):
                # chunk stride padded to 256 f32 so each 192-col matmul
                # output stays inside one 2KB PSUM bank
                ps = ppool.tile([128, GRP, 256], f32, tag="ps")
                for g in range(GRP):
                    u = GRP * k + g           # global chunk index
                    s = u // NCH_S            # shard / partition group
                    p0 = 32 * s
                    wc = SUB * u - WTS * s          # wt col within shard
                    rc = SUB * u - WTS * s + WTS    # rt col within shard line
                    nc.tensor.matmul(
                        ps[:, g, 0:BAND],
                        wr[p0:p0 + KROWS, wc:wc + SUB],
                        wr[p0:p0 + KROWS, rc:rc + BAND],
                        start=True, stop=True,
                        tile_position=(p0, 0),
                    )
                return ps

            def emit_consumers(k, ps):
                ob = outt[:, GRP * k:GRP * (k + 1)]
                if PATHS[k] == "b":
                    # ACT drains PSUM -> SBUF fp16 (values >= 0; overflow to
                    # +inf is min-safe); DVE folds in fp16 (2x mode)
                    sa = apool.tile([128, GRP, BAND], f16, tag="sa")
                    nc.scalar.copy(sa[:], ps[:, :, 0:BAND])
                    f1 = apool.tile([128, GRP, H1], f16, tag="f1")
                    f2 = apool.tile([128, GRP, H2], f16, tag="f2")
                    f3 = apool.tile([128, GRP, H3], f16, tag="f3")
                    nc.vector.tensor_tensor(f1[:], sa[:, :, 0:H1],
                                            sa[:, :, H1:BAND], op=MIN)
                    nc.vector.tensor_tensor(f2[:], f1[:, :, 0:H2],
                                            f1[:, :, H2:H1], op=MIN)
                    nc.vector.tensor_tensor(f3[:], f2[:, :, 0:H3],
                                            f2[:, :, H3:H2], op=MIN)
                    nc.vector.tensor_reduce(ob, f3[:], axis=X, op=MIN)
                else:
                    # DVE sub-reduces straight from PSUM: each starts as soon
                    # as its pair of matmuls lands (short tail)
                    for h in range(0, GRP, 2):
                        nc.vector.tensor_reduce(
                            outt[:, GRP * k + h:GRP * k + h + 2],
                            ps[:, h:h + 2, 0:BAND], axis=X, op=MIN)

            NG = NBLK // GRP
            for k in range(NG - 2):
                ps = emit_mms(k)
                emit_consumers(k, ps)
                if k == NG // 2 - 1:
                    nc.sync.dma_start(out_d[:, 0:NBLK // 2],
                                      outt[:, 0:NBLK // 2])
            # last two groups: emit group NG-1's matmuls and its PSUM
            # sub-reduces BEFORE group NG-2's consumer chain so the in-order
            # DVE queue drains the final PSUM tiles right behind the PE
            # instead of stalling behind the fold chain
            ps6 = emit_mms(NG - 2)
            ps7 = emit_mms(NG - 1)
            emit_consumers(NG - 1, ps7)
            emit_consumers(NG - 2, ps6)
            nc.sync.dma_start(out_d[:, NBLK // 2:], outt[:, NBLK // 2:])
    nc.compile()
    return nc


def _morton_key(pts):
    rng = pts.max(0) - pts.min(0)
    q = ((pts - pts.min(0)) / (rng + 1e-9) * 1023).astype(np.uint64)

    def spread(x):
        x = x & np.uint64(0x3FF)
        x = (x | (x << np.uint64(16))) & np.uint64(0x30000FF)
        x = (x | (x << np.uint64(8))) & np.uint64(0x300F00F)
        x = (x | (x << np.uint64(4))) & np.uint64(0x30C30C3)
        x = (x | (x << np.uint64(2))) & np.uint64(0x9249249)
        return x

    return (spread(q[:, 0]) | (spread(q[:, 1]) << np.uint64(1))
            | (spread(q[:, 2]) << np.uint64(2)))


def _prep_core(X, Y):
    """Host prep for one (batch, direction): returns in_map plus the metadata
    needed to verify and assemble the result."""
    xo = np.argsort(X[:, AXIS], kind="stable")
    yo = np.argsort(Y[:, AXIS], kind="stable")
    Xs = X[xo]
    Ys = Y[yo]
    X2 = (Xs.astype(np.float64) ** 2).sum(1)
    Y2 = (Ys.astype(np.float64) ** 2).sum(1)
    zx = Xs[:, AXIS].astype(np.float64)
    zy = Ys[:, AXIS].astype(np.float64)

    # gap to nearest excluded Y along the sort axis, per query
    i = np.arange(M)
    c = i // SUB
    lo_pos = SUB * c - GUARD         # first included Y position
    hi_pos = SUB * c + SUB + GUARD   # first excluded upper position
    gap = np.full(M, np.inf)
    has_lo = lo_pos > 0
    gap[has_lo] = zx[has_lo] - zy[lo_pos[has_lo] - 1]
    has_hi = hi_pos < P
    gap[has_hi] = np.minimum(gap[has_hi], zy[hi_pos[has_hi]] - zx[has_hi])
    gap = np.maximum(gap, 0.0)

    # conservative NN-distance upper bound via Morton-order neighbors
    allpts = np.concatenate([Xs, Ys]).astype(np.float64)
    mk = _morton_key(allpts)
    inv = np.empty(2 * M, dtype=np.int64)
    inv[np.argsort(mk, kind="stable")] = np.arange(2 * M)
    y_rank = inv[M:]
    order_y = np.argsort(y_rank, kind="stable")
    sorted_ranks = y_rank[order_y]
    K = 16
    idx = np.searchsorted(sorted_ranks, inv[:M])
    cand = np.clip(idx[:, None] + np.arange(-K, K)[None, :], 0, M - 1)
    cands = order_y[cand]
    d2 = ((Xs[:, None, :].astype(np.float64) - Ys[cands].astype(np.float64)) ** 2).sum(-1)
    d_cap2 = d2.min(1)

    hard = np.flatnonzero(~(d_cap2 <= (gap * gap) * 0.98))

    # fp16 hi/lo decomposition of SCALE*X and SCALE*Y; device computes
    # SCALE^2 * (|x|^2 - 2 x.y + |y|^2) in fp32 PSUM via K=13 rows:
    #   r0-2: -2*a_d * c_d     r3-5: -2*a_d * e_d     r6-8: -2*b_d * c_d
    #   r9:   1 * w_hi         r10:  1 * w_lo
    #   r11:  v_hi * 1         r12:  v_lo * 1
    # where a+b ~ SCALE*x, c+e ~ SCALE*y, w_hi+w_lo ~ |SCALE*y|^2,
    # v_hi+v_lo ~ |SCALE*x|^2.
    Xss = (SCALE * Xs).astype(np.float64)
    Yss = (SCALE * Ys).astype(np.float64)
    a = Xss.astype(np.float16)
    bb = (Xss - a.astype(np.float64)).astype(np.float16)
    cc = Yss.astype(np.float16)
    e = (Yss - cc.astype(np.float64)).astype(np.float16)
    w = (Yss ** 2).sum(1)
    wh = w.astype(np.float16)
    wl = (w - wh.astype(np.float64)).astype(np.float16)
    v = (Xss ** 2).sum(1)
    vh = v.astype(np.float16)
    vl = (v - vh.astype(np.float64)).astype(np.float16)

    na = (-2.0 * a.astype(np.float64)).astype(np.float16)  # exact: x2 of fp16
    nb = (-2.0 * bb.astype(np.float64)).astype(np.float16)

    wt = np.empty((KROWS, M), dtype=np.float16)
    wt[0:3, :] = na.T
    wt[3:6, :] = na.T
    wt[6:9, :] = nb.T
    wt[9:11, :] = 1.0
    wt[11, :] = vh
    wt[12, :] = vl

    rt = np.empty((KROWS, P + 2 * PAD), dtype=np.float16)
    ccT = cc.T
    eeT = e.T
    # edge-duplicate padding: repeats of the first/last sorted reference
    # point — real candidates, can never lower a min below the true min.
    for cols, sl in ((slice(0, PAD), 0), (slice(PAD + P, P + 2 * PAD), P - 1)):
        rt[0:3, cols] = ccT[:, sl:sl + 1]
        rt[3:6, cols] = eeT[:, sl:sl + 1]
        rt[6:9, cols] = ccT[:, sl:sl + 1]
        rt[9, cols] = wh[sl]
        rt[10, cols] = wl[sl]
    rt[0:3, PAD:PAD + P] = ccT
    rt[3:6, PAD:PAD + P] = eeT
    rt[6:9, PAD:PAD + P] = ccT
    rt[9, PAD:PAD + P] = wh
    rt[10, PAD:PAD + P] = wl
    rt[11:13, :] = 1.0

    # shard layout: partition group 32s holds wt cols [2048s, 2048(s+1)) and
    # rt cols [2048s, 2048s+2144) (rt in padded coords; windows for chunks
    # 16s..16s+15 fall inside because of the 96-col overlap)
    wr = np.zeros((128, COLS), dtype=np.float16)
    for s in range(NSHARD):
        wr[32 * s:32 * s + KROWS, 0:WTS] = wt[:, WTS * s:WTS * (s + 1)]
        wr[32 * s:32 * s + KROWS, WTS:COLS] = rt[:, WTS * s:WTS * s + RTS]

    return {"wr": wr}, {
        "Xs": Xs.astype(np.float64), "Ys": Ys.astype(np.float64),
        "X2": X2, "Y2": Y2, "gap": gap, "hard": hard,
    }


def _exact_rows(meta, idx):
    """Exact NN distance (float64) for query rows idx against all of Y."""
    Xb = meta["Xs"][idx]
    D = meta["X2"][idx][:, None] + meta["Y2"][None, :] - 2.0 * (Xb @ meta["Ys"].T)
    return D.min(axis=1)


def _post_core(out, meta):
    """Combine device output into sum over queries of min-D (float64)."""
    inv_s2 = 1.0 / (SCALE * SCALE)
    dmin = out.T.reshape(M).astype(np.float64) * inv_s2

    if len(meta["hard"]):
        dmin[meta["hard"]] = _exact_rows(meta, meta["hard"])

    # soundness check for window-only points: device numeric margin includes
    # the bf16 rounding of intermediates (rel ~8e-3) plus fp16 product noise
    g2 = meta["gap"] * meta["gap"]
    ok = dmin <= g2 - 2e-3 * inv_s2 - 8e-3 * np.abs(dmin)
    ok[meta["hard"]] = True
    bad = np.flatnonzero(~ok)
    if len(bad):
        dmin[bad] = _exact_rows(meta, bad)
    if os.environ.get("CHAMFER_DEBUG"):
        print(f"  host-recomputed: hard={len(meta['hard'])} bad={len(bad)}")
    return dmin.sum()


def _install_axon_profile_hook():
    """Make trace=True work under axon when the image's antenv lacks
    axon_hooks: inject a shim module wired to the ctypes NTFF driver."""
    import sys
    import types
    try:
        from antenv.axon_hooks import get_axon_ntff_profile_hook  # noqa: F401
        return
    except ImportError:
        pass
    try:
        import antenv
        from trn_agent_boot.trn_boot import _ntff_profile_via_ctypes
        hook = _ntff_profile_via_ctypes("/opt/axon/libaxon_pjrt.so")
    except Exception:
        hook = None
    mod = types.ModuleType("antenv.axon_hooks")
    state = {"h": hook}
    mod.get_axon_ntff_profile_hook = lambda: state["h"]
    mod.set_axon_ntff_profile_hook = lambda h: state.__setitem__("h", h)
    sys.modules["antenv.axon_hooks"] = mod
    try:
        antenv.axon_hooks = mod
    except Exception:
        pass


def kernel(x_hat, points, likelihoods):
    from concourse.bass_utils import run_bass_kernel_spmd
    global LAST_RESULTS

    trace = bool(int(os.environ.get("CHAMFER_TRACE", "0")))
    if trace:
        _install_axon_profile_hook()

    if "nc" not in _CACHE:
        _CACHE["nc"] = _build_bass()
    nc = _CACHE["nc"]

    in_maps, metas = [], []
    for core in range(8):
        b, d = core // 2, core % 2
        X = x_hat[b] if d == 0 else points[b]
        Y = points[b] if d == 0 else x_hat[b]
        m, meta = _prep_core(np.asarray(X), np.asarray(Y))
        in_maps.append(m)
        metas.append(meta)

    res = run_bass_kernel_spmd(
        nc, in_maps, core_ids=list(range(8)), trace=trace,
    )
    LAST_RESULTS = res

    sums = [_post_core(res.results[c]["out"], metas[c]) for c in range(8)]
    cham_x = sum(sums[c] for c in range(8) if c % 2 == 0) / (B * M)
    cham_y = sum(sums[c] for c in range(8) if c % 2 == 1) / (B * P)
    rec = cham_x + cham_y

    lik = np.asarray(likelihoods, dtype=np.float64)
    bpp = np.log2(lik).sum() / (-(B * P))

    loss = bpp + LMBDA * rec
    return np.array([loss, bpp, rec], dtype=np.float32)
